# revision 42
# baseline (speedup 1.0000x reference)
"""Trainium2 Bass kernel for the BmmEnsemble (ANI-style ensemble MLP) problem.

Reference computation: for each atom n with species s=sp[n],
  atom_e[n] = mean_e( MLP_{s,e}(aev[n]) )   (4-layer CELU MLP, alpha=0.1)
  energy = sum_n atom_e[n]

Strategy (moe_routing):
  * Host: sort atoms by species, distribute each species' atoms evenly over
    the 8 cores, pad each (core, species) block to a fixed capacity with
    zero columns (zero aev + zero bias -> exactly zero contribution), and
    pre-transpose activations to feature-major xT [384, Ptot] so the device
    needs no gather/transpose.
  * Device (identical NEFF on 8 cores): per species, run the E=8 ensemble
    MLPs on that species' atom tiles only (4x FLOP saving vs reference).
    Feature-major layout throughout: weights are the stationary matmul
    operand, atoms stream as the moving operand.
  * CELU(z) = max(z, min(alpha*e^{z/alpha}, alpha) - alpha)  -- exact, and
    safe under overflow of the exp (inf is clamped by the min; g is kept in
    bf16 so its exponent range matches fp32).
    Per chunk: ScalarE Exp (PSUM->SBUF bf16), VectorE dual-op tensor_scalar
    (min, subtract; 16-bit 4x mode), VectorE tensor_tensor max (PSUM fp32 x
    SBUF bf16 -> fp16).
  * Matmul operands (aev, weights, activations) are fp16 (single-pass PE,
    fp32 PSUM accumulation): measured end-to-end rel err ~1e-4.
  * Layer 4 + ensemble mean fold into one dot per atom accumulated in PSUM
    over e; per-tile free-dim reduce accumulates the energy.
  * Host: sum the 8 per-core partial energies.
"""

import numpy as np

_S, _E, _D = 4, 8, 384
_H1, _H2, _H3 = 160, 128, 96
_ALPHA = 0.1
_NC = 8
_TW = 512  # max moving-operand width (fp32)

TRACE = False           # set True (module-level) to capture an NTFF profile
LAST = {}               # filled with exec_time_ns / trace path after a TRACE run

_BUILD_CACHE = {}


def _tile_widths(cap):
    out = []
    rem = cap
    while rem > 0:
        w = min(_TW, rem)
        out.append(w)
        rem -= w
    return out


def _build(caps, dtype_np=np.float32, debug_layers=4, debug_l2_es=None):
    """Build the Bass program for per-species per-core capacities `caps`.

    debug_layers<4 truncates the net after that many layers (debug only):
    the remaining layers are skipped and the per-tile reduce runs on the
    last computed psum chunk.
    """
    import concourse.bass as bass  # noqa: F401
    import concourse.mybir as mybir
    import concourse.tile as tile
    from concourse import bacc

    f32 = mybir.dt.float32
    f16 = mybir.dt.float16
    bf16 = mybir.dt.bfloat16
    Ptot = int(sum(caps))
    widths = [_tile_widths(c) for c in caps]
    ntiles_total = sum(len(ws) for ws in widths)

    nc = bacc.Bacc("TRN2", target_bir_lowering=False, debug=False, num_devices=_NC)

    xT = nc.dram_tensor("xT", [_D, Ptot], f16, kind="ExternalInput").ap()
    w1 = nc.dram_tensor("w1cat", [_S, _D, _E * _H1], f16, kind="ExternalInput").ap()
    # host-side zero-padded L2 weight blocks: block p of ensemble e holds the
    # weight rows for h1 chunk (2e*128+p*128... see l2_split), aligned to the
    # chunk's partitions, zeros elsewhere
    w2 = nc.dram_tensor("w2p", [_S, _E, 2, 128, _H2], f16, kind="ExternalInput").ap()
    w3 = nc.dram_tensor("w3", [_S, _E, _H2, _H3], f16, kind="ExternalInput").ap()
    w4 = nc.dram_tensor("w4m", [_S, _H3, _E], f16, kind="ExternalInput").ap()
    out = nc.dram_tensor("out", [1, 1], f32, kind="ExternalOutput").ap()

    LN_A = float(np.log(_ALPHA))
    INV_A = float(1.0 / _ALPHA)
    A = float(_ALPHA)
    mm_max = mybir.AluOpType.max
    op_min = mybir.AluOpType.min
    op_sub = mybir.AluOpType.subtract
    op_add = mybir.AluOpType.add
    EXP = mybir.ActivationFunctionType.Exp
    X = mybir.AxisListType.X

    # L2 contraction: ensemble e contracts h1 features [160e, 160e+160)
    # stored across 128-row h1 chunks. Mixing matmul base partitions inside
    # one PSUM accumulation group breaks on HW, so every pass contracts a
    # FULL 128-row chunk at base 0 with the unused weight rows zeroed
    # (pass cost depends on moving columns only, so this is free).
    # seg = (h1 chunk, w2 row offset, data_lo, data_hi)
    l2_split = []
    for e in range(_E):
        f0 = e * _H1
        m0, off = divmod(f0, 128)
        len1 = 128 - off
        rem = _H1 - len1
        l2_split.append([(m0, 0, off, 128), (m0 + 1, len1, 0, rem)])

    n_m1 = (_E * _H1) // 128  # 10 chunks of L1 output features

    with tile.TileContext(nc) as tc:
        with (
            tc.tile_pool(name="wpool", bufs=2) as wpool,
            tc.tile_pool(name="w2pool", bufs=32) as w2pool,
            tc.tile_pool(name="h1pool", bufs=22) as h1pool,
            tc.tile_pool(name="h2pool", bufs=10) as h2pool,
            tc.tile_pool(name="h3pool", bufs=10) as h3pool,
            tc.tile_pool(name="gpool", bufs=8) as gpool,
            tc.tile_pool(name="mpool", bufs=8) as mpool,
            tc.tile_pool(name="singles", bufs=1) as singles,
            tc.tile_pool(name="p1", bufs=3, space="PSUM") as p1pool,
            tc.tile_pool(name="p2", bufs=2, space="PSUM") as p2pool,
            tc.tile_pool(name="p3", bufs=2, space="PSUM") as p3pool,
            tc.tile_pool(name="p4", bufs=1, space="PSUM") as p4pool,
        ):
            accs = singles.tile([1, ntiles_total], f32)
            final = singles.tile([1, 1], f32)
            bias_lna = singles.tile([128, 1], f32)
            nc.vector.memset(bias_lna, LN_A)

            # PE warm-up spin: tiny accumulating matmuls on memset data run
            # during the startup DMAs so the HAM clock-gate opens (K=8/8)
            # before the first real L1 burst. ~110ns each, ~6.6us total.
            warm_lhs = singles.tile([1, 1], f16)
            nc.vector.memset(warm_lhs, 0.0)
            warm_rhs = singles.tile([1, 256], f16)
            nc.vector.memset(warm_rhs, 0.0)
            warm_pt = p4pool.tile([1, 256], f32, tag="p4")
            for i in range(60):
                nc.tensor.matmul(
                    warm_pt, warm_lhs, warm_rhs, start=(i == 0), stop=(i == 59)
                )

            # whole xT kept in SBUF; DMA'd in per-species slices (emitted
            # just before each species' tiles) so the first matmuls don't
            # wait for the full transfer
            x_all = singles.tile([128, 3, Ptot], f16)

            def celu(psum_chunk, p, wdt, h_tile):
                g = gpool.tile([p, wdt], bf16, tag="g")
                nc.scalar.activation(
                    out=g, in_=psum_chunk, func=EXP, bias=bias_lna[:p], scale=INV_A
                )
                m = mpool.tile([p, wdt], bf16, tag="m")
                nc.vector.tensor_scalar(
                    out=m, in0=g, scalar1=A, scalar2=A, op0=op_min, op1=op_sub
                )
                nc.vector.tensor_tensor(out=h_tile, in0=psum_chunk, in1=m, op=mm_max)

            # Emission is software-pipelined one tile deep: tile t's layers
            # 2-4 are emitted after tile t+1's layer 1, so the scheduler
            # prioritizes keeping the L1 matmul stream (and its psum
            # evictions) running while the previous tile's tail drains.
            pending_tail = None
            ti = 0
            col0 = 0
            for s in range(_S):
                if caps[s] == 0:
                    continue
                # ---- per-species activations slice + weights ----
                for k in range(3):
                    nc.sync.dma_start(
                        out=x_all[:, k, col0 : col0 + caps[s]],
                        in_=xT[:, col0 : col0 + caps[s]].rearrange(
                            "(k p) n -> p k n", p=128
                        )[:, k, :],
                    )
                w1_sb = wpool.tile([128, 3, _E * _H1], f16, tag="w1")
                nc.sync.dma_start(
                    out=w1_sb, in_=w1[s].rearrange("(k p) m -> p k m", p=128)
                )
                w2_sb = []
                for e in range(_E):
                    pieces = []
                    for pi in range(2):
                        t = w2pool.tile([128, _H2], f16, tag="w2")
                        nc.sync.dma_start(out=t, in_=w2[s, e, pi])
                        pieces.append(t)
                    w2_sb.append(pieces)
                w3_sb = wpool.tile([128, _E, _H3], f16, tag="w3")
                nc.sync.dma_start(out=w3_sb, in_=w3[s].rearrange("e p f -> p e f"))
                w4_sb = wpool.tile([_H3, _E], f16, tag="w4")
                nc.sync.dma_start(out=w4_sb, in_=w4[s])

                for w in widths[s]:
                    # ---- layer 1 (tile t) interleaved with the tail pieces
                    # of tile t-1 so PE demand stays dense ----
                    c0 = col0
                    h1_tiles = []
                    for m in range(n_m1):
                        pt = p1pool.tile([128, w], f32, tag="p1")
                        for k in range(3):
                            nc.tensor.matmul(
                                pt,
                                w1_sb[:, k, m * 128 : (m + 1) * 128],
                                x_all[:, k, c0 : c0 + w],
                                start=(k == 0),
                                stop=(k == 2),
                            )
                        h = h1pool.tile([128, w], f16, tag="h1")
                        celu(pt, 128, w, h)
                        h1_tiles.append(h)
                        if pending_tail is not None:
                            pending_tail(m)

                    if pending_tail is not None:
                        pending_tail(n_m1)

                    def make_tail(h1_tiles, w2_sb, w3_sb, w4_sb, w, ti):
                        h2_tiles = []
                        h3_tiles = []
                        n_l2 = _E if debug_layers >= 2 else 0
                        if debug_l2_es is not None:
                            n_l2 = debug_l2_es
                        n_l3 = _E if debug_layers >= 3 else 0

                        def emit_l2(e):
                            pt = p2pool.tile([128, w], f32, tag="p2")
                            passes = l2_split[e]
                            for pi, (mi, _ro, _dlo, _dhi) in enumerate(passes):
                                nc.tensor.matmul(
                                    pt,
                                    w2_sb[e][pi],
                                    h1_tiles[mi],
                                    start=(pi == 0),
                                    stop=(pi == len(passes) - 1),
                                )
                            h = h2pool.tile([128, w], f16, tag="h2")
                            celu(pt, 128, w, h)
                            h2_tiles.append(h)

                        def emit_l3(e):
                            pt = p3pool.tile([_H3, w], f32, tag="p3")
                            nc.tensor.matmul(
                                pt, w3_sb[:, e, :], h2_tiles[e], start=True, stop=True
                            )
                            h = h3pool.tile([_H3, w], f16, tag="h3")
                            celu(pt, _H3, w, h)
                            h3_tiles.append(h)

                        def emit_l4():
                            pt4 = p4pool.tile([1, w], f32, tag="p4")
                            if debug_layers >= 4:
                                for e in range(_E):
                                    nc.tensor.matmul(
                                        pt4,
                                        w4_sb[:, e : e + 1],
                                        h3_tiles[e],
                                        start=(e == 0),
                                        stop=(e == _E - 1),
                                    )
                            elif debug_layers == 3:
                                nc.tensor.matmul(
                                    pt4, w4_sb[:, 0:1], h3_tiles[0],
                                    start=True, stop=True,
                                )
                            elif debug_layers == 2:
                                nc.tensor.matmul(
                                    pt4, w3_sb[:, 0, 0:1], h2_tiles[0],
                                    start=True, stop=True,
                                )
                            else:
                                nc.tensor.matmul(
                                    pt4, w3_sb[:, 0, 0:1], h1_tiles[0],
                                    start=True, stop=True,
                                )
                            nc.vector.tensor_reduce(
                                out=accs[0:1, ti : ti + 1], in_=pt4, axis=X, op=op_add
                            )

                        def tail(step):
                            # step m in [0, n_m1]: called after L1 chunk m of
                            # the NEXT tile is emitted (or with n_m1 to flush)
                            if step >= n_m1:
                                while len(h2_tiles) < n_l2:
                                    emit_l2(len(h2_tiles))
                                while len(h3_tiles) < n_l3:
                                    emit_l3(len(h3_tiles))
                                emit_l4()
                                return
                            if step < n_l2 and len(h2_tiles) == step:
                                emit_l2(step)
                            if step >= 2 and step - 2 < n_l3 and len(h3_tiles) == step - 2:
                                emit_l3(step - 2)
                        return tail

                    pending_tail = make_tail(h1_tiles, w2_sb, w3_sb, w4_sb, w, ti)
                    ti += 1
                    col0 += w

            if pending_tail is not None:
                pending_tail(n_m1)

            nc.vector.tensor_reduce(
                out=final, in_=accs[0:1, 0:ntiles_total], axis=X, op=op_add
            )
            nc.sync.dma_start(out=out, in_=final)

    nc.finalize()
    return nc


def _pack_w2(W2):
    """Zero-padded, chunk-aligned L2 weight blocks (see _build's l2_split)."""
    w2p = np.zeros((_S, _E, 2, 128, _H2), np.float16)
    for e in range(_E):
        off = (e * _H1) % 128
        len1 = 128 - off
        rem = _H1 - len1
        w2p[:, e, 0, off:128, :] = W2[:, e, 0:len1, :]
        w2p[:, e, 1, 0:rem, :] = W2[:, e, len1:_H1, :]
    return w2p


def _numpy_fallback(species, aev, W1, b1, W2, b2, W3, b3, W4, b4):
    """Exact-math host fallback (used only if biases are nonzero, which the
    reference's setup_inputs never produces)."""
    sp = np.asarray(species).reshape(-1).astype(np.int64)
    x = np.asarray(aev).reshape(-1, _D).astype(np.float32)

    def celu(z):
        return np.maximum(z, 0) + np.minimum(
            _ALPHA * np.expm1(np.minimum(z, 0) / _ALPHA), 0
        ).astype(np.float32)

    total = np.float64(0.0)
    for s in range(_S):
        idx = np.where(sp == s)[0]
        if len(idx) == 0:
            continue
        xs = x[idx]
        dot = np.zeros(len(idx), np.float64)
        for e in range(_E):
            h = celu(xs @ W1[s, e] + b1[s, e])
            h = celu(h @ W2[s, e] + b2[s, e])
            h = celu(h @ W3[s, e] + b3[s, e])
            o = h @ W4[s, e] + b4[s, e]
            dot += o[:, 0]
        total += (dot / _E).sum()
    return np.array([[total]], dtype=np.float32)


def kernel(species, aev, W1, b1, W2, b2, W3, b3, W4, b4):
    from concourse import bass_utils

    b_arrs = [np.asarray(b) for b in (b1, b2, b3, b4)]
    if any(np.abs(b).max() > 0 for b in b_arrs):
        return _numpy_fallback(species, aev, W1, b1, W2, b2, W3, b3, W4, b4)

    sp = np.asarray(species).reshape(-1).astype(np.int64)
    x = np.asarray(aev, dtype=np.float32).reshape(-1, _D)
    N = x.shape[0]

    W1 = np.ascontiguousarray(np.asarray(W1, dtype=np.float32))
    W2 = np.ascontiguousarray(np.asarray(W2, dtype=np.float32))
    W3 = np.ascontiguousarray(np.asarray(W3, dtype=np.float32))
    W4 = np.asarray(W4, dtype=np.float32)

    # ---- route atoms: per species, round-robin across cores ----
    per_core_idx = [[None] * _S for _ in range(_NC)]
    caps = []
    for s in range(_S):
        idx = np.where(sp == s)[0]
        maxc = 0
        for c in range(_NC):
            ci = idx[c::_NC]
            per_core_idx[c][s] = ci
            maxc = max(maxc, len(ci))
        caps.append(((maxc + 7) // 8) * 8 if maxc > 0 else 0)
    caps = tuple(caps)
    Ptot = int(sum(caps))

    # ---- pack per-core transposed activations ----
    offs = np.cumsum([0] + list(caps))[:-1]
    in_maps = []
    w1cat = np.ascontiguousarray(
        W1.transpose(0, 2, 1, 3).reshape(_S, _D, _E * _H1).astype(np.float16)
    )
    w2p = _pack_w2(W2)
    w3_16 = W3.astype(np.float16)
    w4m = np.ascontiguousarray(
        (W4[..., 0] / _E).transpose(0, 2, 1).astype(np.float16)
    )  # [S,96,E]
    x16 = x.astype(np.float16)
    for c in range(_NC):
        xTc = np.zeros((_D, Ptot), dtype=np.float16)
        for s in range(_S):
            ci = per_core_idx[c][s]
            if len(ci):
                xTc[:, offs[s] : offs[s] + len(ci)] = x16[ci].T
        in_maps.append(
            {"xT": xTc, "w1cat": w1cat, "w2p": w2p, "w3": w3_16, "w4m": w4m}
        )

    nc = _BUILD_CACHE.get(caps)
    if nc is None:
        nc = _build(caps)
        _BUILD_CACHE[caps] = nc

    res = bass_utils.run_bass_kernel_spmd(
        nc, in_maps, core_ids=list(range(_NC)), trace=TRACE
    )
    if TRACE:
        LAST["exec_time_ns"] = res.exec_time_ns
        LAST["mean_exec_time_ns"] = res.mean_exec_time_ns
        LAST["trace"] = (
            None if res.instructions_and_trace is None else res.instructions_and_trace[1]
        )

    total = np.float64(0.0)
    for c in range(_NC):
        total += np.float64(res.results[c]["out"][0, 0])
    return np.array([[total]], dtype=np.float32)


# revision 43
# speedup vs baseline: 1.0190x; 1.0190x over previous
"""Trainium2 Bass kernel for the BmmEnsemble (ANI-style ensemble MLP) problem.

Reference computation: for each atom n with species s=sp[n],
  atom_e[n] = mean_e( MLP_{s,e}(aev[n]) )   (4-layer CELU MLP, alpha=0.1)
  energy = sum_n atom_e[n]

Strategy (moe_routing):
  * Host: sort atoms by species, distribute each species' atoms evenly over
    the 8 cores, pad each (core, species) block to a fixed capacity with
    zero columns (zero aev + zero bias -> exactly zero contribution), and
    pre-transpose activations to feature-major xT [384, Ptot] so the device
    needs no gather/transpose.
  * Device (identical NEFF on 8 cores): per species, run the E=8 ensemble
    MLPs on that species' atom tiles only (4x FLOP saving vs reference).
    Feature-major layout throughout: weights are the stationary matmul
    operand, atoms stream as the moving operand.
  * CELU(z) = max(z, min(alpha*e^{z/alpha}, alpha) - alpha)  -- exact, and
    safe under overflow of the exp (inf is clamped by the min; g is kept in
    bf16 so its exponent range matches fp32).
    Per chunk: ScalarE Exp (PSUM->SBUF bf16), VectorE dual-op tensor_scalar
    (min, subtract; 16-bit 4x mode), VectorE tensor_tensor max (PSUM fp32 x
    SBUF bf16 -> fp16).
  * Matmul operands (aev, weights, activations) are fp16 (single-pass PE,
    fp32 PSUM accumulation): measured end-to-end rel err ~1e-4.
  * Layer 4 + ensemble mean fold into one dot per atom accumulated in PSUM
    over e; per-tile free-dim reduce accumulates the energy.
  * Host: sum the 8 per-core partial energies.
"""

import numpy as np

_S, _E, _D = 4, 8, 384
_H1, _H2, _H3 = 160, 128, 96
_ALPHA = 0.1
_NC = 8
_TW = 512  # max moving-operand width (fp32)

TRACE = False           # set True (module-level) to capture an NTFF profile
LAST = {}               # filled with exec_time_ns / trace path after a TRACE run

_BUILD_CACHE = {}


def _tile_widths(cap):
    out = []
    rem = cap
    while rem > 0:
        w = min(_TW, rem)
        out.append(w)
        rem -= w
    return out


def _build(caps, dtype_np=np.float32, debug_layers=4, debug_l2_es=None):
    """Build the Bass program for per-species per-core capacities `caps`.

    debug_layers<4 truncates the net after that many layers (debug only):
    the remaining layers are skipped and the per-tile reduce runs on the
    last computed psum chunk.
    """
    import concourse.bass as bass  # noqa: F401
    import concourse.mybir as mybir
    import concourse.tile as tile
    from concourse import bacc

    f32 = mybir.dt.float32
    f16 = mybir.dt.float16
    bf16 = mybir.dt.bfloat16
    Ptot = int(sum(caps))
    widths = [_tile_widths(c) for c in caps]
    ntiles_total = sum(len(ws) for ws in widths)

    nc = bacc.Bacc("TRN2", target_bir_lowering=False, debug=False, num_devices=_NC)

    xT = nc.dram_tensor("xT", [_D, Ptot], f16, kind="ExternalInput").ap()
    w1 = nc.dram_tensor("w1cat", [_S, _D, _E * _H1], f16, kind="ExternalInput").ap()
    # host-side zero-padded L2 weight blocks: block p of ensemble e holds the
    # weight rows for h1 chunk (2e*128+p*128... see l2_split), aligned to the
    # chunk's partitions, zeros elsewhere
    w2 = nc.dram_tensor("w2p", [_S, _E, 2, 128, _H2], f16, kind="ExternalInput").ap()
    w3 = nc.dram_tensor("w3", [_S, _E, _H2, _H3], f16, kind="ExternalInput").ap()
    w4 = nc.dram_tensor("w4m", [_S, _H3, _E], f16, kind="ExternalInput").ap()
    out = nc.dram_tensor("out", [1, 1], f32, kind="ExternalOutput").ap()

    LN_A = float(np.log(_ALPHA))
    INV_A = float(1.0 / _ALPHA)
    A = float(_ALPHA)
    mm_max = mybir.AluOpType.max
    op_min = mybir.AluOpType.min
    op_sub = mybir.AluOpType.subtract
    op_add = mybir.AluOpType.add
    EXP = mybir.ActivationFunctionType.Exp
    X = mybir.AxisListType.X

    # L2 contraction: ensemble e contracts h1 features [160e, 160e+160)
    # stored across 128-row h1 chunks. Mixing matmul base partitions inside
    # one PSUM accumulation group breaks on HW, so every pass contracts a
    # FULL 128-row chunk at base 0 with the unused weight rows zeroed
    # (pass cost depends on moving columns only, so this is free).
    # seg = (h1 chunk, w2 row offset, data_lo, data_hi)
    l2_split = []
    for e in range(_E):
        f0 = e * _H1
        m0, off = divmod(f0, 128)
        len1 = 128 - off
        rem = _H1 - len1
        l2_split.append([(m0, 0, off, 128), (m0 + 1, len1, 0, rem)])

    n_m1 = (_E * _H1) // 128  # 10 chunks of L1 output features

    with tile.TileContext(nc) as tc:
        with (
            tc.tile_pool(name="wpool", bufs=2) as wpool,
            tc.tile_pool(name="w2pool", bufs=32) as w2pool,
            tc.tile_pool(name="h1pool", bufs=22) as h1pool,
            tc.tile_pool(name="h2pool", bufs=10) as h2pool,
            tc.tile_pool(name="h3pool", bufs=10) as h3pool,
            tc.tile_pool(name="gpool", bufs=8) as gpool,
            tc.tile_pool(name="mpool", bufs=8) as mpool,
            tc.tile_pool(name="singles", bufs=1) as singles,
            tc.tile_pool(name="p1", bufs=3, space="PSUM") as p1pool,
            tc.tile_pool(name="p2", bufs=2, space="PSUM") as p2pool,
            tc.tile_pool(name="p3", bufs=2, space="PSUM") as p3pool,
            tc.tile_pool(name="p4", bufs=1, space="PSUM") as p4pool,
        ):
            accs = singles.tile([1, ntiles_total], f32)
            final = singles.tile([1, 1], f32)
            bias_lna = singles.tile([128, 1], f32)
            nc.vector.memset(bias_lna, LN_A)

            # whole xT kept in SBUF; DMA'd in per-species slices (emitted
            # just before each species' tiles) so the first matmuls don't
            # wait for the full transfer
            x_all = singles.tile([128, 3, Ptot], f16)

            def celu(psum_chunk, p, wdt, h_tile):
                g = gpool.tile([p, wdt], bf16, tag="g")
                nc.scalar.activation(
                    out=g, in_=psum_chunk, func=EXP, bias=bias_lna[:p], scale=INV_A
                )
                m = mpool.tile([p, wdt], bf16, tag="m")
                nc.vector.tensor_scalar(
                    out=m, in0=g, scalar1=A, scalar2=A, op0=op_min, op1=op_sub
                )
                nc.vector.tensor_tensor(out=h_tile, in0=psum_chunk, in1=m, op=mm_max)

            # Emission is software-pipelined one tile deep: tile t's layers
            # 2-4 are emitted after tile t+1's layer 1, so the scheduler
            # prioritizes keeping the L1 matmul stream (and its psum
            # evictions) running while the previous tile's tail drains.
            pending_tail = None
            ti = 0
            col0 = 0
            for s in range(_S):
                if caps[s] == 0:
                    continue
                # ---- per-species activations slice + weights ----
                for k in range(3):
                    nc.sync.dma_start(
                        out=x_all[:, k, col0 : col0 + caps[s]],
                        in_=xT[:, col0 : col0 + caps[s]].rearrange(
                            "(k p) n -> p k n", p=128
                        )[:, k, :],
                    )
                w1_sb = wpool.tile([128, 3, _E * _H1], f16, tag="w1")
                nc.sync.dma_start(
                    out=w1_sb, in_=w1[s].rearrange("(k p) m -> p k m", p=128)
                )
                w2_sb = []
                for e in range(_E):
                    pieces = []
                    for pi in range(2):
                        t = w2pool.tile([128, _H2], f16, tag="w2")
                        nc.sync.dma_start(out=t, in_=w2[s, e, pi])
                        pieces.append(t)
                    w2_sb.append(pieces)
                w3_sb = wpool.tile([128, _E, _H3], f16, tag="w3")
                nc.sync.dma_start(out=w3_sb, in_=w3[s].rearrange("e p f -> p e f"))
                w4_sb = wpool.tile([_H3, _E], f16, tag="w4")
                nc.sync.dma_start(out=w4_sb, in_=w4[s])

                for w in widths[s]:
                    # ---- layer 1 (tile t) interleaved with the tail pieces
                    # of tile t-1 so PE demand stays dense ----
                    c0 = col0
                    h1_tiles = []
                    for m in range(n_m1):
                        pt = p1pool.tile([128, w], f32, tag="p1")
                        for k in range(3):
                            nc.tensor.matmul(
                                pt,
                                w1_sb[:, k, m * 128 : (m + 1) * 128],
                                x_all[:, k, c0 : c0 + w],
                                start=(k == 0),
                                stop=(k == 2),
                            )
                        h = h1pool.tile([128, w], f16, tag="h1")
                        celu(pt, 128, w, h)
                        h1_tiles.append(h)
                        if pending_tail is not None:
                            pending_tail(m)

                    if pending_tail is not None:
                        pending_tail(n_m1)

                    def make_tail(h1_tiles, w2_sb, w3_sb, w4_sb, w, ti):
                        h2_tiles = []
                        h3_tiles = []
                        n_l2 = _E if debug_layers >= 2 else 0
                        if debug_l2_es is not None:
                            n_l2 = debug_l2_es
                        n_l3 = _E if debug_layers >= 3 else 0

                        def emit_l2(e):
                            pt = p2pool.tile([128, w], f32, tag="p2")
                            passes = l2_split[e]
                            for pi, (mi, _ro, _dlo, _dhi) in enumerate(passes):
                                nc.tensor.matmul(
                                    pt,
                                    w2_sb[e][pi],
                                    h1_tiles[mi],
                                    start=(pi == 0),
                                    stop=(pi == len(passes) - 1),
                                )
                            h = h2pool.tile([128, w], f16, tag="h2")
                            celu(pt, 128, w, h)
                            h2_tiles.append(h)

                        def emit_l3(e):
                            pt = p3pool.tile([_H3, w], f32, tag="p3")
                            nc.tensor.matmul(
                                pt, w3_sb[:, e, :], h2_tiles[e], start=True, stop=True
                            )
                            h = h3pool.tile([_H3, w], f16, tag="h3")
                            celu(pt, _H3, w, h)
                            h3_tiles.append(h)

                        def emit_l4():
                            pt4 = p4pool.tile([1, w], f32, tag="p4")
                            if debug_layers >= 4:
                                for e in range(_E):
                                    nc.tensor.matmul(
                                        pt4,
                                        w4_sb[:, e : e + 1],
                                        h3_tiles[e],
                                        start=(e == 0),
                                        stop=(e == _E - 1),
                                    )
                            elif debug_layers == 3:
                                nc.tensor.matmul(
                                    pt4, w4_sb[:, 0:1], h3_tiles[0],
                                    start=True, stop=True,
                                )
                            elif debug_layers == 2:
                                nc.tensor.matmul(
                                    pt4, w3_sb[:, 0, 0:1], h2_tiles[0],
                                    start=True, stop=True,
                                )
                            else:
                                nc.tensor.matmul(
                                    pt4, w3_sb[:, 0, 0:1], h1_tiles[0],
                                    start=True, stop=True,
                                )
                            nc.vector.tensor_reduce(
                                out=accs[0:1, ti : ti + 1], in_=pt4, axis=X, op=op_add
                            )

                        def tail(step):
                            # step m in [0, n_m1]: called after L1 chunk m of
                            # the NEXT tile is emitted (or with n_m1 to flush)
                            if step >= n_m1:
                                while len(h2_tiles) < n_l2:
                                    emit_l2(len(h2_tiles))
                                while len(h3_tiles) < n_l3:
                                    emit_l3(len(h3_tiles))
                                emit_l4()
                                return
                            if step < n_l2 and len(h2_tiles) == step:
                                emit_l2(step)
                            if step >= 2 and step - 2 < n_l3 and len(h3_tiles) == step - 2:
                                emit_l3(step - 2)
                        return tail

                    pending_tail = make_tail(h1_tiles, w2_sb, w3_sb, w4_sb, w, ti)
                    ti += 1
                    col0 += w

            if pending_tail is not None:
                pending_tail(n_m1)

            nc.vector.tensor_reduce(
                out=final, in_=accs[0:1, 0:ntiles_total], axis=X, op=op_add
            )
            nc.sync.dma_start(out=out, in_=final)

    nc.finalize()
    return nc


def _pack_w2(W2):
    """Zero-padded, chunk-aligned L2 weight blocks (see _build's l2_split)."""
    w2p = np.zeros((_S, _E, 2, 128, _H2), np.float16)
    for e in range(_E):
        off = (e * _H1) % 128
        len1 = 128 - off
        rem = _H1 - len1
        w2p[:, e, 0, off:128, :] = W2[:, e, 0:len1, :]
        w2p[:, e, 1, 0:rem, :] = W2[:, e, len1:_H1, :]
    return w2p


def _numpy_fallback(species, aev, W1, b1, W2, b2, W3, b3, W4, b4):
    """Exact-math host fallback (used only if biases are nonzero, which the
    reference's setup_inputs never produces)."""
    sp = np.asarray(species).reshape(-1).astype(np.int64)
    x = np.asarray(aev).reshape(-1, _D).astype(np.float32)

    def celu(z):
        return np.maximum(z, 0) + np.minimum(
            _ALPHA * np.expm1(np.minimum(z, 0) / _ALPHA), 0
        ).astype(np.float32)

    total = np.float64(0.0)
    for s in range(_S):
        idx = np.where(sp == s)[0]
        if len(idx) == 0:
            continue
        xs = x[idx]
        dot = np.zeros(len(idx), np.float64)
        for e in range(_E):
            h = celu(xs @ W1[s, e] + b1[s, e])
            h = celu(h @ W2[s, e] + b2[s, e])
            h = celu(h @ W3[s, e] + b3[s, e])
            o = h @ W4[s, e] + b4[s, e]
            dot += o[:, 0]
        total += (dot / _E).sum()
    return np.array([[total]], dtype=np.float32)


def kernel(species, aev, W1, b1, W2, b2, W3, b3, W4, b4):
    from concourse import bass_utils

    b_arrs = [np.asarray(b) for b in (b1, b2, b3, b4)]
    if any(np.abs(b).max() > 0 for b in b_arrs):
        return _numpy_fallback(species, aev, W1, b1, W2, b2, W3, b3, W4, b4)

    sp = np.asarray(species).reshape(-1).astype(np.int64)
    x = np.asarray(aev, dtype=np.float32).reshape(-1, _D)
    N = x.shape[0]

    W1 = np.ascontiguousarray(np.asarray(W1, dtype=np.float32))
    W2 = np.ascontiguousarray(np.asarray(W2, dtype=np.float32))
    W3 = np.ascontiguousarray(np.asarray(W3, dtype=np.float32))
    W4 = np.asarray(W4, dtype=np.float32)

    # ---- route atoms: per species, round-robin across cores ----
    per_core_idx = [[None] * _S for _ in range(_NC)]
    caps = []
    for s in range(_S):
        idx = np.where(sp == s)[0]
        maxc = 0
        for c in range(_NC):
            ci = idx[c::_NC]
            per_core_idx[c][s] = ci
            maxc = max(maxc, len(ci))
        caps.append(((maxc + 7) // 8) * 8 if maxc > 0 else 0)
    caps = tuple(caps)
    Ptot = int(sum(caps))

    # ---- pack per-core transposed activations ----
    offs = np.cumsum([0] + list(caps))[:-1]
    in_maps = []
    w1cat = np.ascontiguousarray(
        W1.transpose(0, 2, 1, 3).reshape(_S, _D, _E * _H1).astype(np.float16)
    )
    w2p = _pack_w2(W2)
    w3_16 = W3.astype(np.float16)
    w4m = np.ascontiguousarray(
        (W4[..., 0] / _E).transpose(0, 2, 1).astype(np.float16)
    )  # [S,96,E]
    x16 = x.astype(np.float16)
    for c in range(_NC):
        xTc = np.zeros((_D, Ptot), dtype=np.float16)
        for s in range(_S):
            ci = per_core_idx[c][s]
            if len(ci):
                xTc[:, offs[s] : offs[s] + len(ci)] = x16[ci].T
        in_maps.append(
            {"xT": xTc, "w1cat": w1cat, "w2p": w2p, "w3": w3_16, "w4m": w4m}
        )

    nc = _BUILD_CACHE.get(caps)
    if nc is None:
        nc = _build(caps)
        _BUILD_CACHE[caps] = nc

    res = bass_utils.run_bass_kernel_spmd(
        nc, in_maps, core_ids=list(range(_NC)), trace=TRACE
    )
    if TRACE:
        LAST["exec_time_ns"] = res.exec_time_ns
        LAST["mean_exec_time_ns"] = res.mean_exec_time_ns
        LAST["trace"] = (
            None if res.instructions_and_trace is None else res.instructions_and_trace[1]
        )

    total = np.float64(0.0)
    for c in range(_NC):
        total += np.float64(res.results[c]["out"][0, 0])
    return np.array([[total]], dtype=np.float32)
